# revision 2
# baseline (speedup 1.0000x reference)
"""GRU (CustomRNN) Trainium2 kernel, v2.

Data-parallel over batch (256 -> 8 cores x 32). Key optimizations over the
v1 baseline:
  - Recurrent weights U (and the inject identity) stored in fp8e4m3
    (scaled x16 to stay in the normal range), moving operands stay bf16.
    FWL weight loads run ~2x faster for fp8, and the recurrence is
    weight-load bound.
  - The three gate pre-activations live in one PSUM bank [128, 384]
    (cols = gate*128 + mj*32 + b) seeded by a single identity matmul
    injecting A = x@W + b for all gates (1 instruction vs 3).
  - Software-pipelined A computation: the x@W projections for block ib+1
    are interleaved one psum-tile task per step into block ib's recurrence,
    filling PE idle gaps behind the activation critical path. Implemented
    with a 2x-unrolled hardware loop ping-ponging two A buffers.
  - Gate order [r, z, h] so sigmoid(r) (which gates the htild matmul)
    retires as early as possible.
"""

import numpy as np

import concourse.bass as bass
import concourse.mybir as mybir
import concourse.tile as tile
from concourse import bacc
from concourse.bass import ds
from concourse.bass_utils import run_bass_kernel_spmd

SEQ, BATCH, D_IN, D_HID, D_OUT = 2048, 256, 512, 512, 1000
NCORES = 8
BS = BATCH // NCORES          # 32 batch rows per core
KI = D_IN // 128              # 4 contraction chunks for x@W
KH = D_HID // 128             # 4 contraction chunks for h@U
T_BLK = 64                    # timesteps per block
F32 = mybir.dt.float32
BF16 = mybir.dt.bfloat16
FP8 = mybir.dt.float8e4
AF = mybir.ActivationFunctionType
USCALE = 16.0                 # U/W/bias pre-scale so fp8 stays normal
NCH = 512                     # A-phase psum free-dim chunk


def build_bass(seq=SEQ, t_blk=T_BLK, repeat=1):
    assert seq % t_blk == 0
    nblk = seq // t_blk
    assert nblk % 2 == 0
    CB = t_blk * BS           # x/A columns per block
    tch = NCH // BS           # timesteps per A-phase psum chunk
    nch = CB // NCH           # A-phase chunks per block
    G = 3 * D_HID

    nc = bacc.Bacc(None, target_bir_lowering=False)

    x_d = nc.dram_tensor("xt", [KI, 128, (seq + t_blk) * BS], BF16,
                         kind="ExternalInput")
    w_d = nc.dram_tensor("w", [KI, 128, G], BF16, kind="ExternalInput")
    u_d = nc.dram_tensor("u", [KH, 128, G], FP8, kind="ExternalInput")
    b_d = nc.dram_tensor("bias", [128, 12], F32, kind="ExternalInput")
    i_d = nc.dram_tensor("ident", [128, 128], FP8, kind="ExternalInput")
    wfc_d = nc.dram_tensor("wfc", [KH, 128, D_OUT], F32, kind="ExternalInput")
    out_d = nc.dram_tensor("out", [BS, D_OUT], F32, kind="ExternalOutput")

    with tile.TileContext(nc) as tc:
        with (
            tc.tile_pool(name="const", bufs=1) as constp,
            tc.tile_pool(name="st", bufs=2) as stp,
            tc.tile_pool(name="ps", bufs=2, space="PSUM") as psp,
            tc.tile_pool(name="psA", bufs=2, space="PSUM") as psa,
        ):
            u_sb = constp.tile([128, KH, G], FP8)
            w_sb = constp.tile([128, KI, G], BF16)
            b_sb = constp.tile([128, 12], F32)
            ident = constp.tile([128, 128], FP8)
            for k in range(KH):
                nc.sync.dma_start(u_sb[:, k, :], u_d[k])
            for k in range(KI):
                nc.sync.dma_start(w_sb[:, k, :], w_d[k])
            nc.sync.dma_start(b_sb[:], b_d[:])
            nc.sync.dma_start(ident[:], i_d[:])

            # x and A double buffers (explicit ping-pong across loop phases)
            xa = constp.tile([128, KI, CB], BF16)
            xb = constp.tile([128, KI, CB], BF16)
            aa = constp.tile([128, t_blk, 3, 128], BF16)
            ab = constp.tile([128, t_blk, 3, 128], BF16)

            # ping/pong recurrent state, packed-T [128, 128] (col = 32k + b)
            h0 = constp.tile([128, KH * BS], F32)
            h1 = constp.tile([128, KH * BS], F32)
            h0b = constp.tile([128, KH * BS], BF16)
            h1b = constp.tile([128, KH * BS], BF16)
            nc.vector.memset(h0[:], 0.0)
            nc.vector.memset(h0b[:], 0.0)

            adds_pending = []

            def emit_pending_adds():
                while adds_pending:
                    adds_pending.pop(0)()

            def emit_a_task(g, mj, ci, a_buf, x_buf, idx):
                """x@W + b for one (gate, out-tile, col-chunk) of a block."""
                pa = psa.tile([128, NCH], F32, tag="pa")
                wt = w_sb[:, :, g * D_HID + mj * 128:g * D_HID + (mj + 1) * 128]
                for k in range(KI):
                    nc.tensor.matmul(
                        pa[:],
                        wt[:, k, :],
                        x_buf[:, k, ci * NCH:(ci + 1) * NCH],
                        start=(k == 0),
                        stop=(k == KI - 1),
                    )
                t0 = ci * tch
                a_out = a_buf[:, t0:t0 + tch, g, mj * BS:(mj + 1) * BS]
                bias_ap = b_sb[:, g * 4 + mj:g * 4 + mj + 1]

                def do_add():
                    if idx % 2 == 0:
                        nc.vector.tensor_add(
                            a_out,
                            pa[:].rearrange("p (t b) -> p t b", b=BS),
                            bias_ap[:, :, None].to_broadcast((128, tch, BS)),
                        )
                    else:
                        nc.scalar.add(
                            a_out,
                            pa[:].rearrange("p (t b) -> p t b", b=BS),
                            bias_ap,
                        )

                adds_pending.append(do_add)

            def emit_step(t, a_buf):
                hin = h0 if t % 2 == 0 else h1
                hinb = h0b if t % 2 == 0 else h1b
                hout = h1 if t % 2 == 0 else h0
                houtb = h1b if t % 2 == 0 else h0b

                ps = psp.tile([128, 512], F32, tag="ps")
                # single inject of A for all 3 gates (start of accumulation)
                nc.tensor.matmul(ps[:, 0:384], ident[:], a_buf[:, t],
                                 start=True, stop=False,
                                 skip_group_check=True)
                # r gate (g=0) first, then z (g=1): same moving operand
                for g in (0, 1):
                    off = g * D_HID
                    for mj in range(KH):
                        for k in range(KH):
                            nc.tensor.matmul(
                                ps[:, g * 128 + mj * BS:
                                   g * 128 + (mj + 1) * BS],
                                u_sb[:, k, off + mj * 128:off + (mj + 1) * 128],
                                hinb[:, k * BS:(k + 1) * BS],
                                start=False,
                                stop=(k == KH - 1),
                                skip_group_check=True,
                            )

                r_act = stp.tile([128, KH * BS], BF16, tag="r_act")
                nc.scalar.activation(r_act[:], ps[:, 0:128], AF.Sigmoid,
                                     scale=1.0 / USCALE)
                rh = stp.tile([128, KH * BS], BF16, tag="rh")
                nc.vector.tensor_mul(rh[:], r_act[:], hinb[:])

                z_act = stp.tile([128, KH * BS], F32, tag="z_act")
                nc.scalar.activation(z_act[:], ps[:, 128:256], AF.Sigmoid,
                                     scale=1.0 / USCALE)
                zc_act = stp.tile([128, KH * BS], F32, tag="zc_act")
                nc.scalar.activation(zc_act[:], ps[:, 128:256], AF.Sigmoid,
                                     scale=-1.0 / USCALE)
                # t1 = (1-z)*h, ready before tanh completes
                t1 = stp.tile([128, KH * BS], F32, tag="t1")
                nc.vector.tensor_mul(t1[:], zc_act[:], hin[:])

                off = 2 * D_HID
                for mj in range(KH):
                    for k in range(KH):
                        nc.tensor.matmul(
                            ps[:, 256 + mj * BS:256 + (mj + 1) * BS],
                            u_sb[:, k, off + mj * 128:off + (mj + 1) * 128],
                            rh[:, k * BS:(k + 1) * BS],
                            start=False,
                            stop=(k == KH - 1),
                            skip_group_check=True,
                        )

                ht = stp.tile([128, KH * BS], F32, tag="ht")
                nc.scalar.activation(ht[:], ps[:, 256:384], AF.Tanh,
                                     scale=1.0 / USCALE)
                t2 = stp.tile([128, KH * BS], F32, tag="t2")
                nc.vector.tensor_mul(t2[:], z_act[:], ht[:])
                # bf16 state first: it feeds the next step's matmuls
                nc.vector.tensor_add(houtb[:], t1[:], t2[:])
                nc.vector.tensor_add(hout[:], t1[:], t2[:])

            def emit_phase(t_blk_, a_cur, a_nxt, x_nxt):
                """64 recurrence steps on a_cur + interleaved A tasks -> a_nxt."""
                tasks = [(g, mj, ci)
                         for ci in range(nch)
                         for g in range(3)
                         for mj in range(KH)]
                ntask = len(tasks)          # 48
                done = 0
                for t in range(t_blk_):
                    emit_step(t, a_cur)
                    want = min(ntask, max(0, (t - 3)) * ntask // (t_blk_ - 10))
                    while done < want:
                        emit_pending_adds()
                        g, mj, ci = tasks[done]
                        emit_a_task(g, mj, ci, a_nxt, x_nxt, done)
                        done += 1
                while done < ntask:
                    emit_pending_adds()
                    g, mj, ci = tasks[done]
                    emit_a_task(g, mj, ci, a_nxt, x_nxt, done)
                    done += 1
                emit_pending_adds()

            # ---- prologue: x block 0 -> A block 0 (into aa) ----
            nc.sync.dma_start(
                xb[:], x_d[:, :, 0:CB].rearrange("k q c -> q k c"))
            for idx, (g, mj, ci) in enumerate(
                    [(g, mj, ci) for ci in range(nch)
                     for g in range(3) for mj in range(KH)]):
                emit_pending_adds()
                emit_a_task(g, mj, ci, aa, xb, idx)
            emit_pending_adds()

            # ---- main loop: 2 blocks per body ----
            def emit_body(ib):
                # even phase: steps on aa, compute ab = A[ib+1] from xb
                nc.sync.dma_start(
                    xb[:],
                    x_d[:, :, ds(ib * CB + CB, CB)].rearrange("k q c -> q k c"))
                emit_phase(t_blk, aa, ab, xb)
                # odd phase: steps on ab, compute aa = A[ib+2] from xa
                nc.sync.dma_start(
                    xa[:],
                    x_d[:, :, ds(ib * CB + 2 * CB, CB)].rearrange(
                        "k q c -> q k c"))
                emit_phase(t_blk, ab, aa, xa)

            if repeat == 1:
                with tc.For_i(0, nblk, 2,
                              hint_engines=(mybir.EngineType.PE,)) as ib:
                    emit_body(ib)
            else:
                with tc.For_i(0, repeat, 1) as _rep:
                    with tc.For_i(0, nblk, 2,
                                  hint_engines=(mybir.EngineType.PE,)) as ib:
                        emit_body(ib)

            # ---- fc head: relu(h) @ Wfc ----
            wfc_sb = constp.tile([128, KH, D_OUT], F32)
            for k in range(KH):
                nc.sync.dma_start(wfc_sb[:, k, :], wfc_d[k])
            hrelu = stp.tile([128, KH * BS], F32, tag="hrelu")
            nc.scalar.activation(hrelu[:], h0[:], AF.Relu)
            out_sb = stp.tile([BS, D_OUT], F32, tag="outsb")
            for ci in range(2):
                n0, nsz = ci * 500, 500
                po = psa.tile([128, NCH], F32, tag="pa")
                for k in range(KH):
                    nc.tensor.matmul(
                        po[:BS, :nsz],
                        hrelu[:, k * BS:(k + 1) * BS],
                        wfc_sb[:, k, n0:n0 + nsz],
                        start=(k == 0),
                        stop=(k == KH - 1),
                    )
                nc.vector.tensor_copy(out_sb[:, n0:n0 + nsz], po[:BS, :nsz])
            nc.sync.dma_start(out_d[:], out_sb[:])

    nc.finalize()
    return nc


def _prep_inputs(x, Wz, Uz, Wr, Ur, Wh, Uh, bz, buz, br, bur, bh, buh, Wfc,
                 t_blk=T_BLK):
    import ml_dtypes
    seq = x.shape[0]
    # x[t, 32c+b, 128k+q] -> xt[c][k, q, t*32+b], padded with one zero block
    xr = x.reshape(seq, NCORES, BS, KI, 128).transpose(1, 3, 4, 0, 2)
    xt = np.ascontiguousarray(
        xr.astype(ml_dtypes.bfloat16)).reshape(NCORES, KI, 128, seq * BS)
    pad = np.zeros((NCORES, KI, 128, t_blk * BS), ml_dtypes.bfloat16)
    xt = np.concatenate([xt, pad], axis=3)

    # gate order [r, z, h]; W/bias pre-scaled by USCALE to match fp8 U
    w_all = np.concatenate([Wr, Wz, Wh], axis=1) * USCALE   # [512, 1536]
    u_all = np.concatenate([Ur, Uz, Uh], axis=1) * USCALE
    w_dev = np.ascontiguousarray(
        w_all.reshape(KI, 128, 3 * D_HID)).astype(ml_dtypes.bfloat16)
    u_dev = np.ascontiguousarray(
        u_all.reshape(KH, 128, 3 * D_HID)).astype(ml_dtypes.float8_e4m3)
    b_all = np.stack([br + bur, bz + buz, bh + buh]) * USCALE  # [3, 512]
    b_dev = np.ascontiguousarray(
        b_all.reshape(3, 4, 128).transpose(2, 0, 1).reshape(128, 12))
    i_dev = np.eye(128, dtype=np.float32).astype(ml_dtypes.float8_e4m3)
    wfc_dev = np.ascontiguousarray(Wfc.reshape(KH, 128, D_OUT))
    return xt, w_dev, u_dev, b_dev, i_dev, wfc_dev


def make_in_maps(inputs, seq=SEQ, t_blk=T_BLK):
    f = lambda k: np.ascontiguousarray(np.asarray(inputs[k], dtype=np.float32))
    x = f("x")[:seq]
    xt, w_dev, u_dev, b_dev, i_dev, wfc_dev = _prep_inputs(
        x, f("Wz"), f("Uz"), f("Wr"), f("Ur"), f("Wh"), f("Uh"),
        f("bz"), f("buz"), f("br"), f("bur"), f("bh"), f("buh"), f("Wfc"),
        t_blk=t_blk)
    return [
        {"xt": xt[c], "w": w_dev, "u": u_dev, "bias": b_dev, "ident": i_dev,
         "wfc": wfc_dev}
        for c in range(NCORES)
    ]


def run_gru(inputs, seq=SEQ, t_blk=T_BLK, trace=False):
    in_maps = make_in_maps(inputs, seq=seq, t_blk=t_blk)
    nc = build_bass(seq=seq, t_blk=t_blk)
    res = run_bass_kernel_spmd(nc, in_maps, core_ids=list(range(NCORES)),
                               trace=trace)
    logits = np.concatenate([res.results[c]["out"] for c in range(NCORES)], 0)
    logits = logits + np.asarray(inputs["bfc"], np.float32)[None, :]
    m = logits.max(axis=0, keepdims=True)
    lse = m + np.log(np.exp(logits - m).sum(axis=0, keepdims=True))
    out = (logits - lse)[None]
    return out.astype(np.float32), res


def kernel(**inputs) -> np.ndarray:
    out, _ = run_gru(inputs, seq=SEQ, t_blk=T_BLK)
    return out
